# revision 31
# baseline (speedup 1.0000x reference)
"""Trainium2 Bass kernel for nn_Block_9457517985872 (dense transformer block
with linear attention). Token-sharded across 8 NeuronCores: core c handles
batch c//2, sequence half c%2 (2048 tokens). Only cross-core communication is
a pairwise AllReduce of the per-head (kv, ksum) statistics [16,64,65] f32.

Attention path (qkv/proj + attn internals) runs in fp8e4m3 with DoubleRow
matmuls; the MLP runs in bf16. LayerNorm rstd is computed with Newton-Raphson
on the vector engine so the whole kernel needs a single activation-table
switch (Exp set for phase 1, Gelu set for phase 2).

Self-contained: hardcodes all shapes from the problem spec.
"""
import numpy as np
import ml_dtypes
from contextlib import ExitStack

import concourse.bass as bass
import concourse.tile as tile
from concourse import bacc, mybir
from concourse.bass_utils import run_bass_kernel_spmd
from concourse.masks import make_identity

F32 = mybir.dt.float32
BF16 = mybir.dt.bfloat16
F8 = mybir.dt.float8e4
AF = mybir.ActivationFunctionType
ALU = mybir.AluOpType
DR = mybir.MatmulPerfMode.DoubleRow

B, N, C = 4, 4096, 1024
H, D = 16, 64
HID = 4096
TOK = 2048          # tokens per core
NT = TOK // 128     # 16 token tiles
NG = TOK // 512     # 4 token groups
EPS_LN = 1e-5
EPS_ATTN = 1e-6
WS = 32.0           # fp8 weight scale
KVS = 64.0          # kv/ksum fp8 scale (cancels between z and attn)

_BUILD_CACHE = {}


def _emit_ln_stats(nc, pool, x_t, mvq, j):
    """bn_stats/aggr for one 128-token tile into quad slot j of mvq [128,4,2]."""
    stats = pool.tile([128, 2, 6], F32, tag="ln_stats")
    for sg in range(2):
        nc.vector.bn_stats(out=stats[:, sg, :], in_=x_t[:, sg * 512:(sg + 1) * 512])
    nc.vector.bn_aggr(out=mvq[:, j, :], in_=stats[:])


def _emit_ln_nr(nc, pool, mvq):
    """Batched Newton-Raphson rstd for a quad of tiles. mvq [128,4,2] holds
    (mean, var); input var ~= 1.0 so y0=1 converges in 2 iterations. Returns
    nr tile [128,3,4]: row0 = rstd, row1 = -mean*rstd, row2 = tmp."""
    nr = pool.tile([128, 3, 4], F32, tag="ln_nr")
    v = mvq[:, :, 1]
    y, nb, tmp = nr[:, 0, :], nr[:, 1, :], nr[:, 2, :]
    nc.vector.tensor_scalar(out=y, in0=v, scalar1=-0.5,
                            scalar2=1.5 - 0.5 * EPS_LN, op0=ALU.mult, op1=ALU.add)
    for _ in range(2):
        nc.vector.tensor_tensor(out=tmp, in0=y, in1=y, op=ALU.mult)
        nc.vector.scalar_tensor_tensor(out=tmp, in0=tmp, scalar=-0.5, in1=v,
                                       op0=ALU.mult, op1=ALU.mult)
        nc.vector.scalar_tensor_tensor(out=y, in0=tmp, scalar=1.5, in1=y,
                                       op0=ALU.add, op1=ALU.mult)
    nc.vector.tensor_scalar_mul(out=nb, in0=mvq[:, :, 0], scalar1=-1.0)
    nc.vector.tensor_tensor(out=nb, in0=nb, in1=y, op=ALU.mult)
    return nr


def _emit_ln_apply(nc, nr, j, x_t, h_t):
    nc.scalar.activation(out=h_t[:], in_=x_t[:], func=AF.Identity,
                         bias=nr[:, 1, j:j + 1], scale=nr[:, 0, j:j + 1])


def _build(flags, no_cc=False):
    """flags: (has_bq, has_bk, has_bv, has_bg, has_bp, has_b2)"""
    has_bq, has_bk, has_bv, has_bg, has_bp, has_b2 = flags
    nc = bacc.Bacc("TRN2", target_bir_lowering=False, debug=False,
                   num_devices=1 if no_cc else 8)

    xs = nc.dram_tensor("xs", [TOK, C], F32, kind="ExternalInput")
    wq = nc.dram_tensor("wq", [C, C], F8, kind="ExternalInput")        # [c, o] x32
    wkv = nc.dram_tensor("wkv", [C, 2 * C], F8, kind="ExternalInput")  # [c, o] x32
    wp = nc.dram_tensor("wp", [C, C], F8, kind="ExternalInput")        # [c, o] x32
    w1 = nc.dram_tensor("w1", [C, HID], BF16, kind="ExternalInput")
    w2 = nc.dram_tensor("w2", [HID, C], BF16, kind="ExternalInput")
    bq = nc.dram_tensor("bq", [C], F32, kind="ExternalInput")
    bk = nc.dram_tensor("bk", [C], F32, kind="ExternalInput")
    bv = nc.dram_tensor("bv", [C], F32, kind="ExternalInput")
    bg = nc.dram_tensor("bg", [HID], F32, kind="ExternalInput")
    bp = nc.dram_tensor("bp", [C], F32, kind="ExternalInput")
    b2o = nc.dram_tensor("b2o", [C], F32, kind="ExternalInput")
    out = nc.dram_tensor("out", [TOK, C], F32, kind="ExternalOutput")

    xs_v = xs.ap().rearrange("(t p) c -> t p c", p=128)     # [16,128,1024]
    out_v = out.ap().rearrange("(t p) c -> t p c", p=128)
    w1_v = w1.ap().rearrange("(cc p) h -> p cc h", p=128)   # [128,8,4096]
    w2_v = w2.ap().rearrange("(hc p) o -> p hc o", p=128)   # [128,32,1024]

    with tile.TileContext(nc) as tc, ExitStack() as ctx:
        ctx.enter_context(nc.allow_low_precision(
            reason="intentional fp8/bf16 quantized kernel; validated vs reference"))
        const = ctx.enter_context(tc.tile_pool(name="const", bufs=1))
        dram = ctx.enter_context(tc.tile_pool(name="dram", bufs=1, space="DRAM"))
        lnp = ctx.enter_context(tc.tile_pool(name="ln", bufs=2))
        persist = ctx.enter_context(tc.tile_pool(name="persist", bufs=1))

        id_bf = const.tile([128, 128], BF16)
        make_identity(nc, id_bf[:])
        if has_bq:
            bq_sb = const.tile([128, 8], F32)
            nc.sync.dma_start(out=bq_sb[:], in_=bq.ap().rearrange("(oc p) -> p oc", p=128))
        if has_bk:
            bk_bc = const.tile([128, C], F32)
            nc.sync.dma_start(out=bk_bc[:], in_=bass.AP(
                tensor=bk.ap().tensor, offset=0, ap=[[0, 128], [1, C]]))
        if has_bv:
            bv_bc = const.tile([128, C], F32)
            nc.sync.dma_start(out=bv_bc[:], in_=bass.AP(
                tensor=bv.ap().tensor, offset=0, ap=[[0, 128], [1, C]]))
        if has_bg:
            bg_sb = const.tile([128, 32], F32)
            nc.sync.dma_start(out=bg_sb[:], in_=bg.ap().rearrange("(hd p) -> p hd", p=128))
        if has_bp:
            bp_bc = const.tile([128, C], F32)
            nc.sync.dma_start(out=bp_bc[:], in_=bass.AP(
                tensor=bp.ap().tensor, offset=0, ap=[[0, 128], [1, C]]))
        if has_b2:
            b2_bc = const.tile([128, C], F32)
            nc.sync.dma_start(out=b2_bc[:], in_=bass.AP(
                tensor=b2o.ap().tensor, offset=0, ap=[[0, 128], [1, C]]))

        x1s = dram.tile([NT, 128, C], F32)
        z_d = dram.tile([H, TOK], BF16)
        cci = dram.tile([2, 128, 4, 65], F32)
        cco = dram.tile([2, 128, 4, 65], F32)

        # persistent SBUF: full w2 (prefetched early), wp, qT
        w2_sb = persist.tile([128, 32, C], BF16)
        wp_sb = persist.tile([128, 8, C], F8)
        qT = persist.tile([128, 8, TOK], F8)
        # bulk prefetches ride the Activation HWDGE queue so they don't block
        # the phase-1-critical x/wkv/wq loads on the SP queue
        for hc in range(4):
            nc.scalar.dma_start(out=w2_sb[:, 8 * hc:8 * (hc + 1), :],
                                in_=w2_v[:, 8 * hc:8 * (hc + 1), :])
        nc.scalar.dma_start(out=wp_sb[:], in_=wp.ap().rearrange("(cc p) o -> p cc o", p=128))

        # ---------------- Phase 1: LN1, hT, q/k/v, kv+ksum ----------------
        with ExitStack() as p1:
            ep = p1.enter_context
            wkvqp = ep(tc.tile_pool(name="wkvq", bufs=1))
            hTp = ep(tc.tile_pool(name="hTp", bufs=1))
            xinp = ep(tc.tile_pool(name="xin", bufs=5))
            hlocp = ep(tc.tile_pool(name="hloc", bufs=2))
            phip = ep(tc.tile_pool(name="phi", bufs=2))
            kvlocp = ep(tc.tile_pool(name="kvloc", bufs=1))
            kvstp = ep(tc.tile_pool(name="kvst", bufs=1))
            genps = ep(tc.tile_pool(name="gen_ps", bufs=3, space="PSUM"))
            trps = ep(tc.tile_pool(name="tr_ps", bufs=2, space="PSUM"))
            kvps = ep(tc.tile_pool(name="kv_ps", bufs=2, space="PSUM"))
            wkv_sb = wkvqp.tile([128, 8, 2 * C], F8)
            wq_sb = wkvqp.tile([128, 8, C], F8)
            wkv_vv = wkv.ap().rearrange("(cc p) o -> p cc o", p=128)
            for oc in range(2):
                nc.sync.dma_start(out=wkv_sb[:, :, oc * C:(oc + 1) * C],
                                  in_=wkv_vv[:, :, oc * C:(oc + 1) * C])
            nc.sync.dma_start(out=wq_sb[:], in_=wq.ap().rearrange("(cc p) o -> p cc o", p=128))
            hT = hTp.tile([128, 8, TOK], F8)
            k_full = kvlocp.tile([128, NT, C], F8)
            v_full = kvlocp.tile([128, NT, H, 65], F8)
            nc.vector.memset(v_full[:, :, :, 64:65], 1.0)

            for q4 in range(NT // 4):
                mvq = lnp.tile([128, 4, 2], F32, tag="mvq")
                xq = []
                for j in range(4):
                    tt = q4 * 4 + j
                    x_t = xinp.tile([128, C], F32, tag="x", name=f"x_t{tt}")
                    nc.sync.dma_start(out=x_t[:], in_=xs_v[tt])
                    _emit_ln_stats(nc, lnp, x_t, mvq, j)
                    xq.append(x_t)
                nrq = _emit_ln_nr(nc, lnp, mvq)
                for j in range(4):
                    tt = q4 * 4 + j
                    h_t = hlocp.tile([128, C], BF16, tag="h")
                    _emit_ln_apply(nc, nrq, j, xq[j], h_t)
                    # transpose h in bf16 (fp8 PE-transpose needs strided out);
                    # the psum->SBUF copy converts to fp8
                    tr = trps.tile([128, 8, 128], BF16, tag="tr")
                    for cc in range(8):
                        nc.tensor.matmul(tr[:, cc, :], lhsT=h_t[:, cc * 128:(cc + 1) * 128],
                                         rhs=id_bf[:], is_transpose=True,
                                         start=(cc == 0), stop=(cc == 7))
                    nc.vector.tensor_copy(out=hT[:, :, tt * 128:(tt + 1) * 128],
                                          in_=tr[:])
                    hTt = hT[:, :, tt * 128:(tt + 1) * 128]
                    # k (wkv cols 0..1023), v (cols 1024..2047)
                    for oc in range(4):
                        ps = genps.tile([128, 512], F32, tag="gen")
                        for i in range(4):
                            nc.tensor.matmul(ps[:], lhsT=hTt[:, 2 * i:2 * i + 2, :],
                                             rhs=wkv_sb[:, 2 * i:2 * i + 2, oc * 512:(oc + 1) * 512],
                                             start=(i == 0), stop=(i == 3), perf_mode=DR)
                        if oc < 2:   # k: phi = exp(min(w,0)) + max(w,0), w = ps/WS (+bk)
                            wt = phip.tile([128, 512], F32, tag="wt")
                            mt = phip.tile([128, 512], F32, tag="mt")
                            # Act copies psum out scaled; min/exp-prep/max/add
                            # stay in SBUF where Pool can help
                            nc.scalar.activation(out=wt[:], in_=ps[:], func=AF.Identity,
                                                 scale=1.0 / WS)
                            if has_bk:
                                nc.vector.tensor_tensor(out=wt[:], in0=wt[:],
                                                        in1=bk_bc[:, oc * 512:(oc + 1) * 512],
                                                        op=ALU.add)
                            nc.gpsimd.tensor_scalar_min(out=mt[:], in0=wt[:], scalar1=0.0)
                            nc.scalar.activation(out=mt[:], in_=mt[:], func=AF.Exp)
                            nc.gpsimd.tensor_scalar_max(out=wt[:], in0=wt[:], scalar1=0.0)
                            nc.gpsimd.tensor_tensor(out=k_full[:, tt, oc * 512:(oc + 1) * 512],
                                                    in0=wt[:], in1=mt[:], op=ALU.add)
                        else:        # v -> v_full[:, tt, heads, 0:64]
                            h0 = (oc - 2) * 8
                            dst = v_full[:, tt, h0:h0 + 8, 0:64]
                            psv = ps[:].rearrange("p (h d) -> p h d", d=64)
                            if has_bv:
                                vb = bass.AP(tensor=bv.ap().tensor, offset=(oc - 2) * 512,
                                             ap=[[0, 128], [64, 8], [1, 64]])
                                vb_t = phip.tile([128, 8, 64], F32, tag="vb")
                                nc.sync.dma_start(out=vb_t[:], in_=vb)
                                nc.vector.scalar_tensor_tensor(
                                    out=dst, in0=psv, scalar=1.0 / WS, in1=vb_t[:],
                                    op0=ALU.mult, op1=ALU.add)
                            else:
                                nc.vector.tensor_scalar_mul(out=dst, in0=psv,
                                                            scalar1=1.0 / WS)

            # kv[h] = sum_t [k_h]^T @ [v_h | 1]; head pairs (hf=0, hf=1) share
            # a psum bank on disjoint partition halves. Stage -> DRAM -> AllReduce.
            kv_st = kvstp.tile([128, 2, 4, 65], F32)
            for ti in range(2):
                for slot in range(4):
                    kvp = kvps.tile([128, 512], F32, tag="kvacc")
                    for hf in range(2):
                        h = ti * 8 + hf * 4 + slot
                        for t in range(NT):
                            nc.tensor.matmul(
                                kvp[hf * 64:(hf + 1) * 64, 0:65],
                                lhsT=k_full[:, t, h * 64:(h + 1) * 64],
                                rhs=v_full[:, t, h, :],
                                start=(t == 0), stop=(t == NT - 1))
                    nc.vector.tensor_copy(out=kv_st[:, ti, slot, :], in_=kvp[:, 0:65])
                nc.sync.dma_start(out=cci[ti], in_=kv_st[:, ti])
            if no_cc:
                nc.sync.dma_start(out=cco[:], in_=cci[:])
            else:
                nc.gpsimd.collective_compute(
                    "AllReduce", ALU.add,
                    replica_groups=[[0, 1], [2, 3], [4, 5], [6, 7]],
                    ins=[cci[:]], outs=[cco[:]])

            # ---- qT (overlaps the collective): q = phi(h @ wq), transposed ----
            for g in range(NG):
                gsl = slice(g * 512, (g + 1) * 512)
                for oc in range(8):
                    ps = genps.tile([128, 512], F32, tag="gen")
                    for i in range(4):
                        nc.tensor.matmul(ps[:], lhsT=wq_sb[:, 2 * i:2 * i + 2, oc * 128:(oc + 1) * 128],
                                         rhs=hT[:, 2 * i:2 * i + 2, gsl],
                                         start=(i == 0), stop=(i == 3), perf_mode=DR)
                    wt = phip.tile([128, 512], F32, tag="wt")
                    mt = phip.tile([128, 512], F32, tag="mt")
                    if has_bq:
                        nc.scalar.activation(out=wt[:], in_=ps[:], func=AF.Identity,
                                             bias=bq_sb[:, oc:oc + 1], scale=1.0 / WS)
                    else:
                        nc.scalar.activation(out=wt[:], in_=ps[:], func=AF.Identity,
                                             scale=1.0 / WS)
                    nc.gpsimd.tensor_scalar_min(out=mt[:], in0=wt[:], scalar1=0.0)
                    nc.scalar.activation(out=mt[:], in_=mt[:], func=AF.Exp)
                    nc.gpsimd.tensor_scalar_max(out=wt[:], in0=wt[:], scalar1=0.0)
                    nc.gpsimd.tensor_tensor(out=qT[:, oc, gsl], in0=wt[:], in1=mt[:], op=ALU.add)

        # ---------------- Phase 2: attention + proj + LN2 + MLP ----------------
        with ExitStack() as p2:
            ep = p2.enter_context
            kv2p = ep(tc.tile_pool(name="kv2", bufs=1))
            ztp = ep(tc.tile_pool(name="zt", bufs=2))
            zbcp = ep(tc.tile_pool(name="zbc", bufs=1))
            attnp = ep(tc.tile_pool(name="attn", bufs=1))
            xrelp = ep(tc.tile_pool(name="xrel", bufs=2))
            x1tp = ep(tc.tile_pool(name="x1t", bufs=5))
            h2locp = ep(tc.tile_pool(name="h2loc", bufs=2))
            h2Tp = ep(tc.tile_pool(name="h2T", bufs=2))
            w1cp = ep(tc.tile_pool(name="w1c", bufs=3))
            h3p = ep(tc.tile_pool(name="h3p", bufs=1))
            x1relp = ep(tc.tile_pool(name="x1rel", bufs=2))
            outp = ep(tc.tile_pool(name="outp", bufs=1))
            mmps = ep(tc.tile_pool(name="mm_ps", bufs=2, space="PSUM"))
            zps = ep(tc.tile_pool(name="z_ps", bufs=1, space="PSUM"))
            tr2ps = ep(tc.tile_pool(name="tr2_ps", bufs=1, space="PSUM"))
            f1ps = ep(tc.tile_pool(name="f1_ps", bufs=2, space="PSUM"))
            f2ps = ep(tc.tile_pool(name="f2_ps", bufs=2, space="PSUM"))
            # build block-diagonal kv and ksum tiles (scaled 1/KVS) from cco
            kv_stage = kv2p.tile([128, 8, 65], F32)
            kv_bd = kv2p.tile([128, 8, 128], F8)
            bd = kv2p.tile([128, 8, 16], F8)
            nc.gpsimd.memset(kv_bd[:], 0.0)
            nc.gpsimd.memset(bd[:], 0.0)
            for h in range(H):
                ti, hf, slot = h // 8, (h % 8) // 4, h % 4
                pb = (h % 2) * 64
                nc.sync.dma_start(
                    out=kv_stage[pb:pb + 64, h // 2, :],
                    in_=cco[ti, hf * 64:(hf + 1) * 64, slot, :])
                nc.gpsimd.tensor_scalar_mul(
                    out=kv_bd[pb:pb + 64, h // 2, pb:pb + 64],
                    in0=kv_stage[pb:pb + 64, h // 2, 0:64], scalar1=1.0 / KVS)
                nc.gpsimd.tensor_scalar_mul(
                    out=bd[pb:pb + 64, h // 2, h:h + 1],
                    in0=kv_stage[pb:pb + 64, h // 2, 64:65], scalar1=1.0 / KVS)

            h2T_tiles = {}

            def emit_attn_group(g):
                """z, attn, proj(+residual), LN2, h2T for group g."""
                gsl = slice(g * 512, (g + 1) * 512)
                # z = 1/(q . ksum/KVS + eps/KVS)
                zp = zps.tile([16, 512], F32, tag="z")
                for i in range(4):
                    nc.tensor.matmul(zp[:], lhsT=bd[:, 2 * i:2 * i + 2, :],
                                     rhs=qT[:, 2 * i:2 * i + 2, gsl],
                                     start=(i == 0), stop=(i == 3), perf_mode=DR)
                zf = ztp.tile([16, 512], F32, tag="zf")
                zb = ztp.tile([16, 512], BF16, tag="zb")
                nc.vector.tensor_scalar_add(out=zf[:], in0=zp[:], scalar1=EPS_ATTN / KVS)
                nc.vector.reciprocal(out=zb[:], in_=zf[:])
                nc.sync.dma_start(out=z_d[:, gsl], in_=zb[:])
                z_bc = zbcp.tile([128, 8, 512], BF16, tag="zbc")
                zd_ap = z_d[:]
                for sub in range(2):
                    nc.sync.dma_start(
                        out=z_bc[sub * 64:(sub + 1) * 64, :, :],
                        in_=bass.AP(tensor=zd_ap.tensor,
                                    offset=zd_ap.offset + sub * TOK + g * 512,
                                    ap=[[0, 64], [2 * TOK, 8], [1, 512]]))
                # attn_T[cc] = (kv_bd[cc]^T @ qT[cc]) * z
                attn_f8 = attnp.tile([128, 8, 512], F8, tag="attn")
                for cc in range(8):
                    aps = mmps.tile([128, 512], F32, tag="mm")
                    nc.tensor.matmul(aps[:], lhsT=kv_bd[:, cc, :],
                                     rhs=qT[:, cc, gsl], start=True, stop=True)
                    nc.vector.tensor_tensor(out=attn_f8[:, cc, :], in0=aps[:],
                                            in1=z_bc[:, cc, :], op=ALU.mult)
                # proj + residual -> x1; batched LN2 -> h2T group tile
                h2Tg = h2Tp.tile([128, 8, 512], BF16, tag="h2T", name=f"h2T{g}")
                mvq = lnp.tile([128, 4, 2], F32, tag="mvq")
                x1q = []
                for tl in range(4):
                    tt = g * 4 + tl
                    x_rel = xrelp.tile([128, C], F32, tag="xrel")
                    nc.sync.dma_start(out=x_rel[:], in_=xs_v[tt])
                    x1_t = x1tp.tile([128, C], F32, tag="x1")
                    for oc in range(2):
                        osl = slice(oc * 512, (oc + 1) * 512)
                        pps = mmps.tile([128, 512], F32, tag="mm")
                        for i in range(4):
                            nc.tensor.matmul(pps[:], lhsT=attn_f8[:, 2 * i:2 * i + 2, tl * 128:(tl + 1) * 128],
                                             rhs=wp_sb[:, 2 * i:2 * i + 2, osl],
                                             start=(i == 0), stop=(i == 3), perf_mode=DR)
                        nc.vector.scalar_tensor_tensor(
                            out=x1_t[:, osl], in0=pps[:], scalar=1.0 / WS,
                            in1=x_rel[:, osl], op0=ALU.mult, op1=ALU.add)
                        if has_bp:
                            nc.vector.tensor_tensor(out=x1_t[:, osl], in0=x1_t[:, osl],
                                                    in1=bp_bc[:, osl], op=ALU.add)
                    nc.sync.dma_start(out=x1s[tt], in_=x1_t[:])
                    _emit_ln_stats(nc, lnp, x1_t, mvq, tl)
                    x1q.append(x1_t)
                nrq = _emit_ln_nr(nc, lnp, mvq)
                for tl in range(4):
                    h2_t = h2locp.tile([128, C], BF16, tag="h2")
                    _emit_ln_apply(nc, nrq, tl, x1q[tl], h2_t)
                    tr2 = tr2ps.tile([128, 8, 128], BF16, tag="tr2")
                    for cc in range(8):
                        nc.tensor.matmul(tr2[:, cc, :], lhsT=h2_t[:, cc * 128:(cc + 1) * 128],
                                         rhs=id_bf[:], is_transpose=True,
                                         start=(cc == 0), stop=(cc == 7))
                    nc.vector.tensor_copy(out=h2Tg[:, :, tl * 128:(tl + 1) * 128], in_=tr2[:])
                h2T_tiles[g] = h2Tg

            emit_attn_group(0)
            for g in range(NG):
                h2Tg = h2T_tiles.pop(g)
                # fc1 + gelu -> h3 (bf16, hid-major)
                h3 = h3p.tile([128, 32, 512], BF16, tag="h3", name=f"h3_{g}")
                w1pre = {}
                for hd in range(2):
                    w1c = w1cp.tile([128, 8, 128], BF16, tag="w1c", name=f"w1c{g}_{hd}")
                    nc.scalar.dma_start(out=w1c[:], in_=w1_v[:, :, hd * 128:(hd + 1) * 128])
                    w1pre[hd] = w1c
                for hd in range(32):
                    if hd in w1pre:
                        w1c = w1pre.pop(hd)
                    else:
                        w1c = w1cp.tile([128, 8, 128], BF16, tag="w1c", name=f"w1c{g}_{hd}")
                        nc.scalar.dma_start(out=w1c[:], in_=w1_v[:, :, hd * 128:(hd + 1) * 128])
                    fp = f1ps.tile([128, 512], F32, tag="f1")
                    for cc in range(8):
                        nc.tensor.matmul(fp[:], lhsT=w1c[:, cc, :], rhs=h2Tg[:, cc, :],
                                         start=(cc == 0), stop=(cc == 7))
                    if has_bg:
                        nc.scalar.activation(out=h3[:, hd, :], in_=fp[:], func=AF.Gelu,
                                             bias=bg_sb[:, hd:hd + 1], scale=1.0)
                    else:
                        nc.scalar.activation(out=h3[:, hd, :], in_=fp[:], func=AF.Gelu)
                # overlap next group's attention block with this group's fc2
                if g + 1 < NG:
                    emit_attn_group(g + 1)
                # fc2 + residual -> out
                for tl in range(4):
                    tt = g * 4 + tl
                    x1_rel = x1relp.tile([128, C], F32, tag="x1rel")
                    nc.sync.dma_start(out=x1_rel[:], in_=x1s[tt])
                    o_t = outp.tile([128, C], F32, tag="ot")
                    for oc in range(2):
                        osl = slice(oc * 512, (oc + 1) * 512)
                        fp2 = f2ps.tile([128, 512], F32, tag="f2")
                        for hd in range(32):
                            nc.tensor.matmul(fp2[:], lhsT=h3[:, hd, tl * 128:(tl + 1) * 128],
                                             rhs=w2_sb[:, hd, osl],
                                             start=(hd == 0), stop=(hd == 31))
                        nc.vector.tensor_tensor(out=o_t[:, osl], in0=fp2[:],
                                                in1=x1_rel[:, osl], op=ALU.add)
                        if has_b2:
                            nc.vector.tensor_tensor(out=o_t[:, osl], in0=o_t[:, osl],
                                                    in1=b2_bc[:, osl], op=ALU.add)
                    nc.sync.dma_start(out=out_v[tt], in_=o_t[:])

    nc.compile()
    return nc


def _prep_inputs(x, norm1_g, norm1_b, qkv_w, proj_w, proj_b, norm2_g, norm2_b,
                 fc1_w, fc1_b, fc2_w, fc2_b):
    """Host-side weight prep: fold LN gains into weights, LN biases into
    per-output biases; quantize attention weights to fp8 (x32) and MLP
    weights to bf16. Returns (flags, per-core in_maps)."""
    F8NP = ml_dtypes.float8_e4m3
    BFNP = ml_dtypes.bfloat16
    x = np.asarray(x, np.float32)
    g1 = np.asarray(norm1_g, np.float32)
    b1 = np.asarray(norm1_b, np.float32)
    qkv_w = np.asarray(qkv_w, np.float32)
    proj_w = np.asarray(proj_w, np.float32)
    proj_b = np.asarray(proj_b, np.float32)
    g2 = np.asarray(norm2_g, np.float32)
    b2 = np.asarray(norm2_b, np.float32)
    fc1_w = np.asarray(fc1_w, np.float32)
    fc1_b = np.asarray(fc1_b, np.float32)
    fc2_w = np.asarray(fc2_w, np.float32)
    fc2_b = np.asarray(fc2_b, np.float32)

    def f8(w):
        return np.clip(w * WS, -440.0, 440.0).astype(F8NP)

    wq_t = f8(np.ascontiguousarray((qkv_w[0:C] * g1[None, :]).T))
    wkv_t = f8(np.ascontiguousarray((qkv_w[C:3 * C] * g1[None, :]).T))
    wp_t = f8(np.ascontiguousarray(proj_w.T))
    w1_t = np.ascontiguousarray((fc1_w * g2[None, :]).T).astype(BFNP)
    w2_t = np.ascontiguousarray(fc2_w.T).astype(BFNP)
    bq_v = (qkv_w[0:C] @ b1).astype(np.float32)
    bk_v = (qkv_w[C:2 * C] @ b1).astype(np.float32)
    bv_v = (qkv_w[2 * C:3 * C] @ b1).astype(np.float32)
    bg_v = (fc1_w @ b2 + fc1_b).astype(np.float32)

    flags = (bool(np.any(bq_v)), bool(np.any(bk_v)), bool(np.any(bv_v)),
             bool(np.any(bg_v)), bool(np.any(proj_b)), bool(np.any(fc2_b)))

    shared = dict(wq=wq_t, wkv=wkv_t, wp=wp_t, w1=w1_t, w2=w2_t,
                  bq=np.ascontiguousarray(bq_v), bk=np.ascontiguousarray(bk_v),
                  bv=np.ascontiguousarray(bv_v), bg=np.ascontiguousarray(bg_v),
                  bp=proj_b, b2o=fc2_b)
    in_maps = []
    for core in range(8):
        b, half = core // 2, core % 2
        xs = np.ascontiguousarray(x[b, half * TOK:(half + 1) * TOK, :])
        in_maps.append({"xs": xs, **shared})
    return flags, in_maps


def get_compiled(flags):
    if flags not in _BUILD_CACHE:
        _BUILD_CACHE[flags] = _build(flags)
    return _BUILD_CACHE[flags]


def kernel(**inputs) -> np.ndarray:
    flags, in_maps = _prep_inputs(**inputs)
    nc = get_compiled(flags)
    res = run_bass_kernel_spmd(nc, in_maps=in_maps, core_ids=list(range(8)))
    shards = [res.results[c]["out"] for c in range(8)]
    full = np.empty((B, N, C), np.float32)
    for core in range(8):
        b, half = core // 2, core % 2
        full[b, half * TOK:(half + 1) * TOK, :] = shards[core]
    return full


# revision 34
# speedup vs baseline: 1.1196x; 1.1196x over previous
"""Trainium2 Bass kernel for nn_Block_9457517985872 (dense transformer block
with linear attention). Token-sharded across 8 NeuronCores: core c handles
batch c//2, sequence half c%2 (2048 tokens). Only cross-core communication is
a pairwise AllReduce of the per-head (kv, ksum) statistics [16,64,65] f32.

Attention path (qkv/proj + attn internals) runs in fp8e4m3 with DoubleRow
matmuls; the MLP runs in bf16. LayerNorm rstd is computed with Newton-Raphson
on the vector engine so the whole kernel needs a single activation-table
switch (Exp set for phase 1, Gelu set for phase 2).

Self-contained: hardcodes all shapes from the problem spec.
"""
import numpy as np
import ml_dtypes
from contextlib import ExitStack

import concourse.bass as bass
import concourse.tile as tile
from concourse import bacc, mybir
from concourse.bass_utils import run_bass_kernel_spmd
from concourse.masks import make_identity

F32 = mybir.dt.float32
BF16 = mybir.dt.bfloat16
F8 = mybir.dt.float8e4
AF = mybir.ActivationFunctionType
ALU = mybir.AluOpType
DR = mybir.MatmulPerfMode.DoubleRow

B, N, C = 4, 4096, 1024
H, D = 16, 64
HID = 4096
TOK = 2048          # tokens per core
NT = TOK // 128     # 16 token tiles
NG = TOK // 512     # 4 token groups
EPS_LN = 1e-5
EPS_ATTN = 1e-6
WS = 32.0           # fp8 weight scale
KVS = 64.0          # kv/ksum fp8 scale (cancels between z and attn)

_BUILD_CACHE = {}


def _emit_ln_stats(nc, pool, x_t, mvq, j):
    """bn_stats/aggr for one 128-token tile into quad slot j of mvq [128,4,2]."""
    stats = pool.tile([128, 2, 6], F32, tag="ln_stats")
    for sg in range(2):
        nc.vector.bn_stats(out=stats[:, sg, :], in_=x_t[:, sg * 512:(sg + 1) * 512])
    nc.vector.bn_aggr(out=mvq[:, j, :], in_=stats[:])


def _emit_ln_nr(nc, pool, mvq):
    """Batched Newton-Raphson rstd for a quad of tiles. mvq [128,4,2] holds
    (mean, var); input var ~= 1.0 so y0=1 converges in 2 iterations. Returns
    nr tile [128,3,4]: row0 = rstd, row1 = -mean*rstd, row2 = tmp."""
    nr = pool.tile([128, 3, 4], F32, tag="ln_nr")
    v = mvq[:, :, 1]
    y, nb, tmp = nr[:, 0, :], nr[:, 1, :], nr[:, 2, :]
    nc.vector.tensor_scalar(out=y, in0=v, scalar1=-0.5,
                            scalar2=1.5 - 0.5 * EPS_LN, op0=ALU.mult, op1=ALU.add)
    for _ in range(2):
        nc.vector.tensor_tensor(out=tmp, in0=y, in1=y, op=ALU.mult)
        nc.vector.scalar_tensor_tensor(out=tmp, in0=tmp, scalar=-0.5, in1=v,
                                       op0=ALU.mult, op1=ALU.mult)
        nc.vector.scalar_tensor_tensor(out=y, in0=tmp, scalar=1.5, in1=y,
                                       op0=ALU.add, op1=ALU.mult)
    nc.vector.tensor_scalar_mul(out=nb, in0=mvq[:, :, 0], scalar1=-1.0)
    nc.vector.tensor_tensor(out=nb, in0=nb, in1=y, op=ALU.mult)
    return nr


def _emit_ln_apply(nc, nr, j, x_t, h_t):
    nc.scalar.activation(out=h_t[:], in_=x_t[:], func=AF.Identity,
                         bias=nr[:, 1, j:j + 1], scale=nr[:, 0, j:j + 1])


def _build(flags, no_cc=False):
    """flags: (has_bq, has_bk, has_bv, has_bg, has_bp, has_b2)"""
    has_bq, has_bk, has_bv, has_bg, has_bp, has_b2 = flags
    nc = bacc.Bacc("TRN2", target_bir_lowering=False, debug=False,
                   num_devices=1 if no_cc else 8)

    xs = nc.dram_tensor("xs", [TOK, C], F32, kind="ExternalInput")
    wq = nc.dram_tensor("wq", [C, C], F8, kind="ExternalInput")        # [c, o] x32
    wkv = nc.dram_tensor("wkv", [C, 2 * C], F8, kind="ExternalInput")  # [c, o] x32
    wp = nc.dram_tensor("wp", [C, C], F8, kind="ExternalInput")        # [c, o] x32
    w1 = nc.dram_tensor("w1", [C, HID], BF16, kind="ExternalInput")
    w2 = nc.dram_tensor("w2", [HID, C], BF16, kind="ExternalInput")
    bq = nc.dram_tensor("bq", [C], F32, kind="ExternalInput")
    bk = nc.dram_tensor("bk", [C], F32, kind="ExternalInput")
    bv = nc.dram_tensor("bv", [C], F32, kind="ExternalInput")
    bg = nc.dram_tensor("bg", [HID], F32, kind="ExternalInput")
    bp = nc.dram_tensor("bp", [C], F32, kind="ExternalInput")
    b2o = nc.dram_tensor("b2o", [C], F32, kind="ExternalInput")
    out = nc.dram_tensor("out", [TOK, C], F32, kind="ExternalOutput")

    xs_v = xs.ap().rearrange("(t p) c -> t p c", p=128)     # [16,128,1024]
    out_v = out.ap().rearrange("(t p) c -> t p c", p=128)
    w1_v = w1.ap().rearrange("(cc p) h -> p cc h", p=128)   # [128,8,4096]
    w2_v = w2.ap().rearrange("(hc p) o -> p hc o", p=128)   # [128,32,1024]

    with tile.TileContext(nc) as tc, ExitStack() as ctx:
        ctx.enter_context(nc.allow_low_precision(
            reason="intentional fp8/bf16 quantized kernel; validated vs reference"))
        const = ctx.enter_context(tc.tile_pool(name="const", bufs=1))
        dram = ctx.enter_context(tc.tile_pool(name="dram", bufs=1, space="DRAM"))
        lnp = ctx.enter_context(tc.tile_pool(name="ln", bufs=2))
        persist = ctx.enter_context(tc.tile_pool(name="persist", bufs=1))

        id_bf = const.tile([128, 128], BF16)
        make_identity(nc, id_bf[:])
        if has_bq:
            bq_sb = const.tile([128, 8], F32)
            nc.sync.dma_start(out=bq_sb[:], in_=bq.ap().rearrange("(oc p) -> p oc", p=128))
        if has_bk:
            bk_bc = const.tile([128, C], F32)
            nc.sync.dma_start(out=bk_bc[:], in_=bass.AP(
                tensor=bk.ap().tensor, offset=0, ap=[[0, 128], [1, C]]))
        if has_bv:
            bv_bc = const.tile([128, C], F32)
            nc.sync.dma_start(out=bv_bc[:], in_=bass.AP(
                tensor=bv.ap().tensor, offset=0, ap=[[0, 128], [1, C]]))
        if has_bg:
            bg_sb = const.tile([128, 32], F32)
            nc.sync.dma_start(out=bg_sb[:], in_=bg.ap().rearrange("(hd p) -> p hd", p=128))
        if has_bp:
            bp_bc = const.tile([128, C], F32)
            nc.sync.dma_start(out=bp_bc[:], in_=bass.AP(
                tensor=bp.ap().tensor, offset=0, ap=[[0, 128], [1, C]]))
        if has_b2:
            b2_bc = const.tile([128, C], F32)
            nc.sync.dma_start(out=b2_bc[:], in_=bass.AP(
                tensor=b2o.ap().tensor, offset=0, ap=[[0, 128], [1, C]]))

        x1s = dram.tile([NT, 128, C], F32)
        z_d = dram.tile([H, TOK], BF16)
        cci = dram.tile([2, 128, 4, 65], F32)
        cco = dram.tile([2, 128, 4, 65], F32)

        # persistent SBUF: full w2 (prefetched early), wp, qT
        w2_sb = persist.tile([128, 32, C], BF16)
        wp_sb = persist.tile([128, 8, C], F8)
        qT = persist.tile([128, 8, TOK], F8)
        # bulk prefetches ride the Activation HWDGE queue so they don't block
        # the phase-1-critical x/wkv/wq loads on the SP queue
        for hc in range(4):
            nc.scalar.dma_start(out=w2_sb[:, 8 * hc:8 * (hc + 1), :],
                                in_=w2_v[:, 8 * hc:8 * (hc + 1), :])
        nc.scalar.dma_start(out=wp_sb[:], in_=wp.ap().rearrange("(cc p) o -> p cc o", p=128))

        # ---------------- Phase 1: LN1, hT, q/k/v, kv+ksum ----------------
        with ExitStack() as p1:
            ep = p1.enter_context
            wkvqp = ep(tc.tile_pool(name="wkvq", bufs=1))
            hTp = ep(tc.tile_pool(name="hTp", bufs=1))
            xinp = ep(tc.tile_pool(name="xin", bufs=5))
            hlocp = ep(tc.tile_pool(name="hloc", bufs=2))
            phip = ep(tc.tile_pool(name="phi", bufs=2))
            kvlocp = ep(tc.tile_pool(name="kvloc", bufs=1))
            kvstp = ep(tc.tile_pool(name="kvst", bufs=1))
            genps = ep(tc.tile_pool(name="gen_ps", bufs=3, space="PSUM"))
            trps = ep(tc.tile_pool(name="tr_ps", bufs=2, space="PSUM"))
            kvps = ep(tc.tile_pool(name="kv_ps", bufs=2, space="PSUM"))
            wkv_sb = wkvqp.tile([128, 8, 2 * C], F8)
            wq_sb = wkvqp.tile([128, 8, C], F8)
            wkv_vv = wkv.ap().rearrange("(cc p) o -> p cc o", p=128)
            for oc in range(2):
                nc.sync.dma_start(out=wkv_sb[:, :, oc * C:(oc + 1) * C],
                                  in_=wkv_vv[:, :, oc * C:(oc + 1) * C])
            nc.sync.dma_start(out=wq_sb[:], in_=wq.ap().rearrange("(cc p) o -> p cc o", p=128))
            hT = hTp.tile([128, 8, TOK], F8)
            k_full = kvlocp.tile([128, NT, C], F8)
            v_full = kvlocp.tile([128, NT, H, 65], F8)
            nc.vector.memset(v_full[:, :, :, 64:65], 1.0)

            for q4 in range(NT // 4):
                mvq = lnp.tile([128, 4, 2], F32, tag="mvq")
                xq = []
                for j in range(4):
                    tt = q4 * 4 + j
                    x_t = xinp.tile([128, C], F32, tag="x", name=f"x_t{tt}")
                    nc.sync.dma_start(out=x_t[:], in_=xs_v[tt])
                    _emit_ln_stats(nc, lnp, x_t, mvq, j)
                    xq.append(x_t)
                nrq = _emit_ln_nr(nc, lnp, mvq)
                for j in range(4):
                    tt = q4 * 4 + j
                    h_t = hlocp.tile([128, C], BF16, tag="h")
                    _emit_ln_apply(nc, nrq, j, xq[j], h_t)
                    # transpose h in bf16 (fp8 PE-transpose needs strided out);
                    # the psum->SBUF copy converts to fp8
                    tr = trps.tile([128, 8, 128], BF16, tag="tr")
                    for cc in range(8):
                        nc.tensor.matmul(tr[:, cc, :], lhsT=h_t[:, cc * 128:(cc + 1) * 128],
                                         rhs=id_bf[:], is_transpose=True,
                                         start=(cc == 0), stop=(cc == 7))
                    nc.vector.tensor_copy(out=hT[:, :, tt * 128:(tt + 1) * 128],
                                          in_=tr[:])
                    hTt = hT[:, :, tt * 128:(tt + 1) * 128]
                    # k (wkv cols 0..1023), v (cols 1024..2047)
                    for oc in range(4):
                        ps = genps.tile([128, 512], F32, tag="gen")
                        for i in range(4):
                            nc.tensor.matmul(ps[:], lhsT=hTt[:, 2 * i:2 * i + 2, :],
                                             rhs=wkv_sb[:, 2 * i:2 * i + 2, oc * 512:(oc + 1) * 512],
                                             start=(i == 0), stop=(i == 3), perf_mode=DR)
                        if oc < 2:   # k: phi = exp(min(w,0)) + max(w,0), w = ps/WS (+bk)
                            if has_bk:
                                nc.vector.scalar_tensor_tensor(
                                    out=ps[:], in0=ps[:], scalar=1.0 / WS,
                                    in1=bk_bc[:, oc * 512:(oc + 1) * 512], op0=ALU.mult, op1=ALU.add)
                                sc = 1.0
                            else:
                                sc = 1.0 / WS
                            mt = phip.tile([128, 512], F32, tag="mt")
                            rt = phip.tile([128, 512], F32, tag="rt")
                            nc.vector.tensor_scalar_min(out=mt[:], in0=ps[:], scalar1=0.0)
                            nc.scalar.activation(out=mt[:], in_=mt[:], func=AF.Exp, scale=sc)
                            nc.scalar.activation(out=rt[:], in_=ps[:], func=AF.Relu, scale=sc)
                            nc.vector.tensor_tensor(out=k_full[:, tt, oc * 512:(oc + 1) * 512],
                                                    in0=rt[:], in1=mt[:], op=ALU.add)
                        else:        # v -> v_full[:, tt, heads, 0:64]
                            h0 = (oc - 2) * 8
                            dst = v_full[:, tt, h0:h0 + 8, 0:64]
                            psv = ps[:].rearrange("p (h d) -> p h d", d=64)
                            if has_bv:
                                vb = bass.AP(tensor=bv.ap().tensor, offset=(oc - 2) * 512,
                                             ap=[[0, 128], [64, 8], [1, 64]])
                                vb_t = phip.tile([128, 8, 64], F32, tag="vb")
                                nc.sync.dma_start(out=vb_t[:], in_=vb)
                                nc.vector.scalar_tensor_tensor(
                                    out=dst, in0=psv, scalar=1.0 / WS, in1=vb_t[:],
                                    op0=ALU.mult, op1=ALU.add)
                            else:
                                nc.vector.tensor_scalar_mul(out=dst, in0=psv,
                                                            scalar1=1.0 / WS)

            # kv[h] = sum_t [k_h]^T @ [v_h | 1]; head pairs (hf=0, hf=1) share
            # a psum bank on disjoint partition halves. Stage -> DRAM -> AllReduce.
            kv_st = kvstp.tile([128, 2, 4, 65], F32)
            for ti in range(2):
                for slot in range(4):
                    kvp = kvps.tile([128, 512], F32, tag="kvacc")
                    for hf in range(2):
                        h = ti * 8 + hf * 4 + slot
                        for t in range(NT):
                            nc.tensor.matmul(
                                kvp[hf * 64:(hf + 1) * 64, 0:65],
                                lhsT=k_full[:, t, h * 64:(h + 1) * 64],
                                rhs=v_full[:, t, h, :],
                                start=(t == 0), stop=(t == NT - 1))
                    nc.vector.tensor_copy(out=kv_st[:, ti, slot, :], in_=kvp[:, 0:65])
                nc.sync.dma_start(out=cci[ti], in_=kv_st[:, ti])
            if no_cc:
                nc.sync.dma_start(out=cco[:], in_=cci[:])
            else:
                nc.gpsimd.collective_compute(
                    "AllReduce", ALU.add,
                    replica_groups=[[0, 1], [2, 3], [4, 5], [6, 7]],
                    ins=[cci[:]], outs=[cco[:]])

            # ---- qT (overlaps the collective): q = phi(h @ wq), transposed ----
            for g in range(NG):
                gsl = slice(g * 512, (g + 1) * 512)
                for oc in range(8):
                    ps = genps.tile([128, 512], F32, tag="gen")
                    for i in range(4):
                        nc.tensor.matmul(ps[:], lhsT=wq_sb[:, 2 * i:2 * i + 2, oc * 128:(oc + 1) * 128],
                                         rhs=hT[:, 2 * i:2 * i + 2, gsl],
                                         start=(i == 0), stop=(i == 3), perf_mode=DR)
                    mt = phip.tile([128, 512], F32, tag="mt")
                    rt = phip.tile([128, 512], F32, tag="rt")
                    if has_bq:
                        bsl = bq_sb[:, oc:oc + 1]
                        nc.vector.tensor_scalar(out=mt[:], in0=ps[:], scalar1=bsl,
                                                scalar2=0.0, op0=ALU.add, op1=ALU.min)
                        nc.scalar.activation(out=mt[:], in_=mt[:], func=AF.Exp, scale=1.0 / WS)
                        nc.scalar.activation(out=rt[:], in_=ps[:], func=AF.Relu,
                                             bias=bsl, scale=1.0 / WS)
                    else:
                        nc.vector.tensor_scalar_min(out=mt[:], in0=ps[:], scalar1=0.0)
                        nc.scalar.activation(out=mt[:], in_=mt[:], func=AF.Exp, scale=1.0 / WS)
                        nc.scalar.activation(out=rt[:], in_=ps[:], func=AF.Relu, scale=1.0 / WS)
                    nc.vector.tensor_tensor(out=qT[:, oc, gsl], in0=rt[:], in1=mt[:], op=ALU.add)

        # ---------------- Phase 2: attention + proj + LN2 + MLP ----------------
        with ExitStack() as p2:
            ep = p2.enter_context
            kv2p = ep(tc.tile_pool(name="kv2", bufs=1))
            ztp = ep(tc.tile_pool(name="zt", bufs=2))
            zbcp = ep(tc.tile_pool(name="zbc", bufs=1))
            attnp = ep(tc.tile_pool(name="attn", bufs=1))
            xrelp = ep(tc.tile_pool(name="xrel", bufs=2))
            x1tp = ep(tc.tile_pool(name="x1t", bufs=5))
            h2locp = ep(tc.tile_pool(name="h2loc", bufs=2))
            h2Tp = ep(tc.tile_pool(name="h2T", bufs=2))
            w1cp = ep(tc.tile_pool(name="w1c", bufs=3))
            h3p = ep(tc.tile_pool(name="h3p", bufs=1))
            x1relp = ep(tc.tile_pool(name="x1rel", bufs=2))
            outp = ep(tc.tile_pool(name="outp", bufs=1))
            mmps = ep(tc.tile_pool(name="mm_ps", bufs=2, space="PSUM"))
            zps = ep(tc.tile_pool(name="z_ps", bufs=1, space="PSUM"))
            tr2ps = ep(tc.tile_pool(name="tr2_ps", bufs=1, space="PSUM"))
            f1ps = ep(tc.tile_pool(name="f1_ps", bufs=2, space="PSUM"))
            f2ps = ep(tc.tile_pool(name="f2_ps", bufs=2, space="PSUM"))
            # build block-diagonal kv and ksum tiles (scaled 1/KVS) from cco
            kv_stage = kv2p.tile([128, 8, 65], F32)
            kv_bd = kv2p.tile([128, 8, 128], F8)
            bd = kv2p.tile([128, 8, 16], F8)
            nc.vector.memset(kv_bd[:], 0.0)
            nc.vector.memset(bd[:], 0.0)
            for h in range(H):
                ti, hf, slot = h // 8, (h % 8) // 4, h % 4
                pb = (h % 2) * 64
                nc.sync.dma_start(
                    out=kv_stage[pb:pb + 64, h // 2, :],
                    in_=cco[ti, hf * 64:(hf + 1) * 64, slot, :])
                nc.vector.tensor_scalar_mul(
                    out=kv_bd[pb:pb + 64, h // 2, pb:pb + 64],
                    in0=kv_stage[pb:pb + 64, h // 2, 0:64], scalar1=1.0 / KVS)
                nc.vector.tensor_scalar_mul(
                    out=bd[pb:pb + 64, h // 2, h:h + 1],
                    in0=kv_stage[pb:pb + 64, h // 2, 64:65], scalar1=1.0 / KVS)

            h2T_tiles = {}

            def emit_attn_group(g):
                """z, attn, proj(+residual), LN2, h2T for group g."""
                gsl = slice(g * 512, (g + 1) * 512)
                # z = 1/(q . ksum/KVS + eps/KVS)
                zp = zps.tile([16, 512], F32, tag="z")
                for i in range(4):
                    nc.tensor.matmul(zp[:], lhsT=bd[:, 2 * i:2 * i + 2, :],
                                     rhs=qT[:, 2 * i:2 * i + 2, gsl],
                                     start=(i == 0), stop=(i == 3), perf_mode=DR)
                zf = ztp.tile([16, 512], F32, tag="zf")
                zb = ztp.tile([16, 512], BF16, tag="zb")
                nc.vector.tensor_scalar_add(out=zf[:], in0=zp[:], scalar1=EPS_ATTN / KVS)
                nc.vector.reciprocal(out=zb[:], in_=zf[:])
                nc.sync.dma_start(out=z_d[:, gsl], in_=zb[:])
                z_bc = zbcp.tile([128, 8, 512], BF16, tag="zbc")
                zd_ap = z_d[:]
                for sub in range(2):
                    nc.sync.dma_start(
                        out=z_bc[sub * 64:(sub + 1) * 64, :, :],
                        in_=bass.AP(tensor=zd_ap.tensor,
                                    offset=zd_ap.offset + sub * TOK + g * 512,
                                    ap=[[0, 64], [2 * TOK, 8], [1, 512]]))
                # attn_T[cc] = (kv_bd[cc]^T @ qT[cc]) * z
                attn_f8 = attnp.tile([128, 8, 512], F8, tag="attn")
                for cc in range(8):
                    aps = mmps.tile([128, 512], F32, tag="mm")
                    nc.tensor.matmul(aps[:], lhsT=kv_bd[:, cc, :],
                                     rhs=qT[:, cc, gsl], start=True, stop=True)
                    nc.vector.tensor_tensor(out=attn_f8[:, cc, :], in0=aps[:],
                                            in1=z_bc[:, cc, :], op=ALU.mult)
                # proj + residual -> x1; batched LN2 -> h2T group tile
                h2Tg = h2Tp.tile([128, 8, 512], BF16, tag="h2T", name=f"h2T{g}")
                mvq = lnp.tile([128, 4, 2], F32, tag="mvq")
                x1q = []
                for tl in range(4):
                    tt = g * 4 + tl
                    x_rel = xrelp.tile([128, C], F32, tag="xrel")
                    nc.sync.dma_start(out=x_rel[:], in_=xs_v[tt])
                    x1_t = x1tp.tile([128, C], F32, tag="x1")
                    for oc in range(2):
                        osl = slice(oc * 512, (oc + 1) * 512)
                        pps = mmps.tile([128, 512], F32, tag="mm")
                        for i in range(4):
                            nc.tensor.matmul(pps[:], lhsT=attn_f8[:, 2 * i:2 * i + 2, tl * 128:(tl + 1) * 128],
                                             rhs=wp_sb[:, 2 * i:2 * i + 2, osl],
                                             start=(i == 0), stop=(i == 3), perf_mode=DR)
                        nc.vector.scalar_tensor_tensor(
                            out=x1_t[:, osl], in0=pps[:], scalar=1.0 / WS,
                            in1=x_rel[:, osl], op0=ALU.mult, op1=ALU.add)
                        if has_bp:
                            nc.vector.tensor_tensor(out=x1_t[:, osl], in0=x1_t[:, osl],
                                                    in1=bp_bc[:, osl], op=ALU.add)
                    nc.sync.dma_start(out=x1s[tt], in_=x1_t[:])
                    _emit_ln_stats(nc, lnp, x1_t, mvq, tl)
                    x1q.append(x1_t)
                nrq = _emit_ln_nr(nc, lnp, mvq)
                for tl in range(4):
                    h2_t = h2locp.tile([128, C], BF16, tag="h2")
                    _emit_ln_apply(nc, nrq, tl, x1q[tl], h2_t)
                    tr2 = tr2ps.tile([128, 8, 128], BF16, tag="tr2")
                    for cc in range(8):
                        nc.tensor.matmul(tr2[:, cc, :], lhsT=h2_t[:, cc * 128:(cc + 1) * 128],
                                         rhs=id_bf[:], is_transpose=True,
                                         start=(cc == 0), stop=(cc == 7))
                    nc.vector.tensor_copy(out=h2Tg[:, :, tl * 128:(tl + 1) * 128], in_=tr2[:])
                h2T_tiles[g] = h2Tg

            emit_attn_group(0)
            for g in range(NG):
                h2Tg = h2T_tiles.pop(g)
                # fc1 + gelu -> h3 (bf16, hid-major)
                h3 = h3p.tile([128, 32, 512], BF16, tag="h3", name=f"h3_{g}")
                w1pre = {}
                for hd in range(2):
                    w1c = w1cp.tile([128, 8, 128], BF16, tag="w1c", name=f"w1c{g}_{hd}")
                    nc.scalar.dma_start(out=w1c[:], in_=w1_v[:, :, hd * 128:(hd + 1) * 128])
                    w1pre[hd] = w1c
                for hd in range(32):
                    if hd in w1pre:
                        w1c = w1pre.pop(hd)
                    else:
                        w1c = w1cp.tile([128, 8, 128], BF16, tag="w1c", name=f"w1c{g}_{hd}")
                        nc.scalar.dma_start(out=w1c[:], in_=w1_v[:, :, hd * 128:(hd + 1) * 128])
                    fp = f1ps.tile([128, 512], F32, tag="f1")
                    for cc in range(8):
                        nc.tensor.matmul(fp[:], lhsT=w1c[:, cc, :], rhs=h2Tg[:, cc, :],
                                         start=(cc == 0), stop=(cc == 7))
                    if has_bg:
                        nc.scalar.activation(out=h3[:, hd, :], in_=fp[:], func=AF.Gelu,
                                             bias=bg_sb[:, hd:hd + 1], scale=1.0)
                    else:
                        nc.scalar.activation(out=h3[:, hd, :], in_=fp[:], func=AF.Gelu)
                # overlap next group's attention block with this group's fc2
                if g + 1 < NG:
                    emit_attn_group(g + 1)
                # fc2 + residual -> out
                for tl in range(4):
                    tt = g * 4 + tl
                    x1_rel = x1relp.tile([128, C], F32, tag="x1rel")
                    nc.sync.dma_start(out=x1_rel[:], in_=x1s[tt])
                    o_t = outp.tile([128, C], F32, tag="ot")
                    for oc in range(2):
                        osl = slice(oc * 512, (oc + 1) * 512)
                        fp2 = f2ps.tile([128, 512], F32, tag="f2")
                        for hd in range(32):
                            nc.tensor.matmul(fp2[:], lhsT=h3[:, hd, tl * 128:(tl + 1) * 128],
                                             rhs=w2_sb[:, hd, osl],
                                             start=(hd == 0), stop=(hd == 31))
                        nc.vector.tensor_tensor(out=o_t[:, osl], in0=fp2[:],
                                                in1=x1_rel[:, osl], op=ALU.add)
                        if has_b2:
                            nc.vector.tensor_tensor(out=o_t[:, osl], in0=o_t[:, osl],
                                                    in1=b2_bc[:, osl], op=ALU.add)
                    nc.sync.dma_start(out=out_v[tt], in_=o_t[:])

    nc.compile()
    return nc


def _prep_inputs(x, norm1_g, norm1_b, qkv_w, proj_w, proj_b, norm2_g, norm2_b,
                 fc1_w, fc1_b, fc2_w, fc2_b):
    """Host-side weight prep: fold LN gains into weights, LN biases into
    per-output biases; quantize attention weights to fp8 (x32) and MLP
    weights to bf16. Returns (flags, per-core in_maps)."""
    F8NP = ml_dtypes.float8_e4m3
    BFNP = ml_dtypes.bfloat16
    x = np.asarray(x, np.float32)
    g1 = np.asarray(norm1_g, np.float32)
    b1 = np.asarray(norm1_b, np.float32)
    qkv_w = np.asarray(qkv_w, np.float32)
    proj_w = np.asarray(proj_w, np.float32)
    proj_b = np.asarray(proj_b, np.float32)
    g2 = np.asarray(norm2_g, np.float32)
    b2 = np.asarray(norm2_b, np.float32)
    fc1_w = np.asarray(fc1_w, np.float32)
    fc1_b = np.asarray(fc1_b, np.float32)
    fc2_w = np.asarray(fc2_w, np.float32)
    fc2_b = np.asarray(fc2_b, np.float32)

    def f8(w):
        return np.clip(w * WS, -440.0, 440.0).astype(F8NP)

    wq_t = f8(np.ascontiguousarray((qkv_w[0:C] * g1[None, :]).T))
    wkv_t = f8(np.ascontiguousarray((qkv_w[C:3 * C] * g1[None, :]).T))
    wp_t = f8(np.ascontiguousarray(proj_w.T))
    w1_t = np.ascontiguousarray((fc1_w * g2[None, :]).T).astype(BFNP)
    w2_t = np.ascontiguousarray(fc2_w.T).astype(BFNP)
    bq_v = (qkv_w[0:C] @ b1).astype(np.float32)
    bk_v = (qkv_w[C:2 * C] @ b1).astype(np.float32)
    bv_v = (qkv_w[2 * C:3 * C] @ b1).astype(np.float32)
    bg_v = (fc1_w @ b2 + fc1_b).astype(np.float32)

    flags = (bool(np.any(bq_v)), bool(np.any(bk_v)), bool(np.any(bv_v)),
             bool(np.any(bg_v)), bool(np.any(proj_b)), bool(np.any(fc2_b)))

    shared = dict(wq=wq_t, wkv=wkv_t, wp=wp_t, w1=w1_t, w2=w2_t,
                  bq=np.ascontiguousarray(bq_v), bk=np.ascontiguousarray(bk_v),
                  bv=np.ascontiguousarray(bv_v), bg=np.ascontiguousarray(bg_v),
                  bp=proj_b, b2o=fc2_b)
    in_maps = []
    for core in range(8):
        b, half = core // 2, core % 2
        xs = np.ascontiguousarray(x[b, half * TOK:(half + 1) * TOK, :])
        in_maps.append({"xs": xs, **shared})
    return flags, in_maps


def get_compiled(flags):
    if flags not in _BUILD_CACHE:
        _BUILD_CACHE[flags] = _build(flags)
    return _BUILD_CACHE[flags]


def kernel(**inputs) -> np.ndarray:
    flags, in_maps = _prep_inputs(**inputs)
    nc = get_compiled(flags)
    res = run_bass_kernel_spmd(nc, in_maps=in_maps, core_ids=list(range(8)))
    shards = [res.results[c]["out"] for c in range(8)]
    full = np.empty((B, N, C), np.float32)
    for core in range(8):
        b, half = core // 2, core % 2
        full[b, half * TOK:(half + 1) * TOK, :] = shards[core]
    return full


# revision 39
# speedup vs baseline: 1.2019x; 1.0734x over previous
"""Trainium2 Bass kernel for nn_Block_9457517985872 (dense transformer block
with linear attention). Token-sharded across 8 NeuronCores: core c handles
batch c//2, sequence half c%2 (2048 tokens). Only cross-core communication is
a pairwise AllReduce of the per-head (kv, ksum) statistics [16,64,65] f32.

Attention path (qkv/proj + attn internals) runs in fp8e4m3 with DoubleRow
matmuls; the MLP runs in bf16. LayerNorm rstd is computed with Newton-Raphson
on the vector engine so the whole kernel needs a single activation-table
switch (Exp set for phase 1, Gelu set for phase 2).

Self-contained: hardcodes all shapes from the problem spec.
"""
import numpy as np
import ml_dtypes
from contextlib import ExitStack

import concourse.bass as bass
import concourse.tile as tile
from concourse import bacc, mybir
from concourse.bass_utils import run_bass_kernel_spmd
from concourse.masks import make_identity

F32 = mybir.dt.float32
BF16 = mybir.dt.bfloat16
F8 = mybir.dt.float8e4
AF = mybir.ActivationFunctionType
ALU = mybir.AluOpType
DR = mybir.MatmulPerfMode.DoubleRow

B, N, C = 4, 4096, 1024
H, D = 16, 64
HID = 4096
TOK = 2048          # tokens per core
NT = TOK // 128     # 16 token tiles
NG = TOK // 512     # 4 token groups
EPS_LN = 1e-5
EPS_ATTN = 1e-6
WS = 32.0           # fp8 weight scale
KVS = 64.0          # kv/ksum fp8 scale (cancels between z and attn)

_BUILD_CACHE = {}


def _emit_ln_stats(nc, pool, x_t, mvq, j):
    """bn_stats/aggr for one 128-token tile into quad slot j of mvq [128,4,2]."""
    stats = pool.tile([128, 2, 6], F32, tag="ln_stats")
    for sg in range(2):
        nc.vector.bn_stats(out=stats[:, sg, :], in_=x_t[:, sg * 512:(sg + 1) * 512])
    nc.vector.bn_aggr(out=mvq[:, j, :], in_=stats[:])


def _emit_ln_nr(nc, pool, mvq):
    """Batched Newton-Raphson rstd for a quad of tiles. mvq [128,4,2] holds
    (mean, var); input var ~= 1.0 so y0=1 converges in 2 iterations. Returns
    nr tile [128,3,4]: row0 = rstd, row1 = -mean*rstd, row2 = tmp."""
    nr = pool.tile([128, 3, 4], F32, tag="ln_nr")
    v = mvq[:, :, 1]
    y, nb, tmp = nr[:, 0, :], nr[:, 1, :], nr[:, 2, :]
    nc.vector.tensor_scalar(out=y, in0=v, scalar1=-0.5,
                            scalar2=1.5 - 0.5 * EPS_LN, op0=ALU.mult, op1=ALU.add)
    for _ in range(2):
        nc.vector.tensor_tensor(out=tmp, in0=y, in1=y, op=ALU.mult)
        nc.vector.scalar_tensor_tensor(out=tmp, in0=tmp, scalar=-0.5, in1=v,
                                       op0=ALU.mult, op1=ALU.mult)
        nc.vector.scalar_tensor_tensor(out=y, in0=tmp, scalar=1.5, in1=y,
                                       op0=ALU.add, op1=ALU.mult)
    nc.vector.tensor_scalar_mul(out=nb, in0=mvq[:, :, 0], scalar1=-1.0)
    nc.vector.tensor_tensor(out=nb, in0=nb, in1=y, op=ALU.mult)
    return nr


def _emit_ln_apply(nc, nr, j, x_t, h_t):
    nc.scalar.activation(out=h_t[:], in_=x_t[:], func=AF.Identity,
                         bias=nr[:, 1, j:j + 1], scale=nr[:, 0, j:j + 1])


def _build(flags, no_cc=False):
    """flags: (has_bq, has_bk, has_bv, has_bg, has_bp, has_b2)"""
    has_bq, has_bk, has_bv, has_bg, has_bp, has_b2 = flags
    nc = bacc.Bacc("TRN2", target_bir_lowering=False, debug=False,
                   num_devices=1 if no_cc else 8)

    xs = nc.dram_tensor("xs", [TOK, C], F32, kind="ExternalInput")
    wq = nc.dram_tensor("wq", [C, C], F8, kind="ExternalInput")        # [c, o] x32
    wkv = nc.dram_tensor("wkv", [C, 2 * C], F8, kind="ExternalInput")  # [c, o] x32
    wp = nc.dram_tensor("wp", [C, C], F8, kind="ExternalInput")        # [c, o] x32
    w1 = nc.dram_tensor("w1", [C, HID], BF16, kind="ExternalInput")
    w2 = nc.dram_tensor("w2", [HID, C], BF16, kind="ExternalInput")
    bq = nc.dram_tensor("bq", [C], F32, kind="ExternalInput")
    bk = nc.dram_tensor("bk", [C], F32, kind="ExternalInput")
    bv = nc.dram_tensor("bv", [C], F32, kind="ExternalInput")
    bg = nc.dram_tensor("bg", [HID], F32, kind="ExternalInput")
    bp = nc.dram_tensor("bp", [C], F32, kind="ExternalInput")
    b2o = nc.dram_tensor("b2o", [C], F32, kind="ExternalInput")
    out = nc.dram_tensor("out", [TOK, C], F32, kind="ExternalOutput")

    xs_v = xs.ap().rearrange("(t p) c -> t p c", p=128)     # [16,128,1024]
    out_v = out.ap().rearrange("(t p) c -> t p c", p=128)
    w1_v = w1.ap().rearrange("(cc p) h -> p cc h", p=128)   # [128,8,4096]
    w2_v = w2.ap().rearrange("(hc p) o -> p hc o", p=128)   # [128,32,1024]

    with tile.TileContext(nc) as tc, ExitStack() as ctx:
        ctx.enter_context(nc.allow_low_precision(
            reason="intentional fp8/bf16 quantized kernel; validated vs reference"))
        const = ctx.enter_context(tc.tile_pool(name="const", bufs=1))
        dram = ctx.enter_context(tc.tile_pool(name="dram", bufs=1, space="DRAM"))
        lnp = ctx.enter_context(tc.tile_pool(name="ln", bufs=2))
        persist = ctx.enter_context(tc.tile_pool(name="persist", bufs=1))

        if has_bq:
            bq_sb = const.tile([128, 8], F32)
            nc.sync.dma_start(out=bq_sb[:], in_=bq.ap().rearrange("(oc p) -> p oc", p=128))
        if has_bk:
            bk_bc = const.tile([128, C], F32)
            nc.sync.dma_start(out=bk_bc[:], in_=bass.AP(
                tensor=bk.ap().tensor, offset=0, ap=[[0, 128], [1, C]]))
        if has_bv:
            bv_bc = const.tile([128, C], F32)
            nc.sync.dma_start(out=bv_bc[:], in_=bass.AP(
                tensor=bv.ap().tensor, offset=0, ap=[[0, 128], [1, C]]))
        if has_bg:
            bg_sb = const.tile([128, 32], F32)
            nc.sync.dma_start(out=bg_sb[:], in_=bg.ap().rearrange("(hd p) -> p hd", p=128))
        if has_bp:
            bp_bc = const.tile([128, C], F32)
            nc.sync.dma_start(out=bp_bc[:], in_=bass.AP(
                tensor=bp.ap().tensor, offset=0, ap=[[0, 128], [1, C]]))
        if has_b2:
            b2_bc = const.tile([128, C], F32)
            nc.sync.dma_start(out=b2_bc[:], in_=bass.AP(
                tensor=b2o.ap().tensor, offset=0, ap=[[0, 128], [1, C]]))

        x1s = dram.tile([NT, 128, C], F32)
        z_d = dram.tile([H, TOK], BF16)
        cci = dram.tile([2, 128, 4, 65], F32)
        cco = dram.tile([2, 128, 4, 65], F32)

        # persistent SBUF: full w2 (prefetched early), wp, qT
        w2_sb = persist.tile([128, 32, C], BF16)
        wp_sb = persist.tile([128, 8, C], F8)
        qT = persist.tile([128, 8, TOK], F8)
        # bulk prefetches ride the Activation HWDGE queue so they don't block
        # the phase-1-critical x/wkv/wq loads on the SP queue
        for hc in range(4):
            nc.scalar.dma_start(out=w2_sb[:, 8 * hc:8 * (hc + 1), :],
                                in_=w2_v[:, 8 * hc:8 * (hc + 1), :])
        nc.scalar.dma_start(out=wp_sb[:], in_=wp.ap().rearrange("(cc p) o -> p cc o", p=128))

        # ---------------- Phase 1: LN1, hT, q/k/v, kv+ksum ----------------
        with ExitStack() as p1:
            ep = p1.enter_context
            wkvqp = ep(tc.tile_pool(name="wkvq", bufs=1))
            hTp = ep(tc.tile_pool(name="hTp", bufs=1))
            xinp = ep(tc.tile_pool(name="xin", bufs=5))
            hlocp = ep(tc.tile_pool(name="hloc", bufs=2))
            phip = ep(tc.tile_pool(name="phi", bufs=2))
            kvlocp = ep(tc.tile_pool(name="kvloc", bufs=1))
            kvstp = ep(tc.tile_pool(name="kvst", bufs=1))
            trsp = ep(tc.tile_pool(name="trs", bufs=2))
            genps = ep(tc.tile_pool(name="gen_ps", bufs=3, space="PSUM"))
            kvps = ep(tc.tile_pool(name="kv_ps", bufs=2, space="PSUM"))
            wkv_sb = wkvqp.tile([128, 8, 2 * C], F8)
            wq_sb = wkvqp.tile([128, 8, C], F8)
            wkv_vv = wkv.ap().rearrange("(cc p) o -> p cc o", p=128)
            for oc in range(2):
                nc.sync.dma_start(out=wkv_sb[:, :, oc * C:(oc + 1) * C],
                                  in_=wkv_vv[:, :, oc * C:(oc + 1) * C])
            nc.sync.dma_start(out=wq_sb[:], in_=wq.ap().rearrange("(cc p) o -> p cc o", p=128))
            hT = hTp.tile([128, 8, TOK], F8)
            k_full = kvlocp.tile([128, NT, C], F8)
            v_full = kvlocp.tile([128, NT, H, 65], F8)
            nc.vector.memset(v_full[:, :, :, 64:65], 1.0)

            for q4 in range(NT // 4):
                mvq = lnp.tile([128, 4, 2], F32, tag="mvq")
                xq = []
                for j in range(4):
                    tt = q4 * 4 + j
                    x_t = xinp.tile([128, C], F32, tag="x", name=f"x_t{tt}")
                    nc.sync.dma_start(out=x_t[:], in_=xs_v[tt])
                    _emit_ln_stats(nc, lnp, x_t, mvq, j)
                    xq.append(x_t)
                nrq = _emit_ln_nr(nc, lnp, mvq)
                for j in range(4):
                    tt = q4 * 4 + j
                    h_t = hlocp.tile([128, C], BF16, tag="h")
                    _emit_ln_apply(nc, nrq, j, xq[j], h_t)
                    # transpose h via the DMA XBAR (keeps the PE free); the
                    # SBUF->SBUF copy converts bf16 -> fp8
                    hTs = trsp.tile([128, 8, 128], BF16, tag="hTs")
                    for cc in range(8):
                        nc.sync.dma_start_transpose(out=hTs[:, cc, :],
                                                    in_=h_t[:, cc * 128:(cc + 1) * 128])
                    nc.vector.tensor_copy(out=hT[:, :, tt * 128:(tt + 1) * 128],
                                          in_=hTs[:])
                    hTt = hT[:, :, tt * 128:(tt + 1) * 128]
                    # k (wkv cols 0..1023), v (cols 1024..2047)
                    for oc in range(4):
                        ps = genps.tile([128, 512], F32, tag="gen")
                        for i in range(4):
                            nc.tensor.matmul(ps[:], lhsT=hTt[:, 2 * i:2 * i + 2, :],
                                             rhs=wkv_sb[:, 2 * i:2 * i + 2, oc * 512:(oc + 1) * 512],
                                             start=(i == 0), stop=(i == 3), perf_mode=DR)
                        if oc < 2:   # k: phi = exp(min(w,0)) + max(w,0), w = ps/WS (+bk)
                            if has_bk:
                                nc.vector.scalar_tensor_tensor(
                                    out=ps[:], in0=ps[:], scalar=1.0 / WS,
                                    in1=bk_bc[:, oc * 512:(oc + 1) * 512], op0=ALU.mult, op1=ALU.add)
                                sc = 1.0
                            else:
                                sc = 1.0 / WS
                            mt = phip.tile([128, 512], F32, tag="mt")
                            rt = phip.tile([128, 512], F32, tag="rt")
                            nc.vector.tensor_scalar_min(out=mt[:], in0=ps[:], scalar1=0.0)
                            nc.scalar.activation(out=mt[:], in_=mt[:], func=AF.Exp, scale=sc)
                            nc.scalar.activation(out=rt[:], in_=ps[:], func=AF.Relu, scale=sc)
                            nc.vector.tensor_tensor(out=k_full[:, tt, oc * 512:(oc + 1) * 512],
                                                    in0=rt[:], in1=mt[:], op=ALU.add)
                        else:        # v -> v_full[:, tt, heads, 0:64]
                            h0 = (oc - 2) * 8
                            dst = v_full[:, tt, h0:h0 + 8, 0:64]
                            psv = ps[:].rearrange("p (h d) -> p h d", d=64)
                            if has_bv:
                                vb = bass.AP(tensor=bv.ap().tensor, offset=(oc - 2) * 512,
                                             ap=[[0, 128], [64, 8], [1, 64]])
                                vb_t = phip.tile([128, 8, 64], F32, tag="vb")
                                nc.sync.dma_start(out=vb_t[:], in_=vb)
                                nc.vector.scalar_tensor_tensor(
                                    out=dst, in0=psv, scalar=1.0 / WS, in1=vb_t[:],
                                    op0=ALU.mult, op1=ALU.add)
                            else:
                                nc.vector.tensor_scalar_mul(out=dst, in0=psv,
                                                            scalar1=1.0 / WS)

            # kv[h] = sum_t [k_h]^T @ [v_h | 1]; head pairs (hf=0, hf=1) share
            # a psum bank on disjoint partition halves. Stage -> DRAM -> AllReduce.
            kv_st = kvstp.tile([128, 2, 4, 65], F32)
            for ti in range(2):
                for slot in range(4):
                    kvp = kvps.tile([128, 512], F32, tag="kvacc")
                    for hf in range(2):
                        h = ti * 8 + hf * 4 + slot
                        for t in range(NT):
                            nc.tensor.matmul(
                                kvp[hf * 64:(hf + 1) * 64, 0:65],
                                lhsT=k_full[:, t, h * 64:(h + 1) * 64],
                                rhs=v_full[:, t, h, :],
                                start=(t == 0), stop=(t == NT - 1))
                    nc.vector.tensor_copy(out=kv_st[:, ti, slot, :], in_=kvp[:, 0:65])
                nc.sync.dma_start(out=cci[ti], in_=kv_st[:, ti])
            if no_cc:
                nc.sync.dma_start(out=cco[:], in_=cci[:])
            else:
                nc.gpsimd.collective_compute(
                    "AllReduce", ALU.add,
                    replica_groups=[[0, 1], [2, 3], [4, 5], [6, 7]],
                    ins=[cci[:]], outs=[cco[:]])

            # ---- qT (overlaps the collective): q = phi(h @ wq), transposed ----
            for g in range(NG):
                gsl = slice(g * 512, (g + 1) * 512)
                for oc in range(8):
                    ps = genps.tile([128, 512], F32, tag="gen")
                    for i in range(4):
                        nc.tensor.matmul(ps[:], lhsT=wq_sb[:, 2 * i:2 * i + 2, oc * 128:(oc + 1) * 128],
                                         rhs=hT[:, 2 * i:2 * i + 2, gsl],
                                         start=(i == 0), stop=(i == 3), perf_mode=DR)
                    mt = phip.tile([128, 512], F32, tag="mt")
                    rt = phip.tile([128, 512], F32, tag="rt")
                    if has_bq:
                        bsl = bq_sb[:, oc:oc + 1]
                        nc.vector.tensor_scalar(out=mt[:], in0=ps[:], scalar1=bsl,
                                                scalar2=0.0, op0=ALU.add, op1=ALU.min)
                        nc.scalar.activation(out=mt[:], in_=mt[:], func=AF.Exp, scale=1.0 / WS)
                        nc.scalar.activation(out=rt[:], in_=ps[:], func=AF.Relu,
                                             bias=bsl, scale=1.0 / WS)
                    else:
                        nc.vector.tensor_scalar_min(out=mt[:], in0=ps[:], scalar1=0.0)
                        nc.scalar.activation(out=mt[:], in_=mt[:], func=AF.Exp, scale=1.0 / WS)
                        nc.scalar.activation(out=rt[:], in_=ps[:], func=AF.Relu, scale=1.0 / WS)
                    nc.vector.tensor_tensor(out=qT[:, oc, gsl], in0=rt[:], in1=mt[:], op=ALU.add)

        # ---------------- Phase 2: attention + proj + LN2 + MLP ----------------
        with ExitStack() as p2:
            ep = p2.enter_context
            kv2p = ep(tc.tile_pool(name="kv2", bufs=1))
            ztp = ep(tc.tile_pool(name="zt", bufs=2))
            zbcp = ep(tc.tile_pool(name="zbc", bufs=1))
            attnp = ep(tc.tile_pool(name="attn", bufs=1))
            xrelp = ep(tc.tile_pool(name="xrel", bufs=2))
            x1tp = ep(tc.tile_pool(name="x1t", bufs=5))
            h2locp = ep(tc.tile_pool(name="h2loc", bufs=2))
            h2Tp = ep(tc.tile_pool(name="h2T", bufs=2))
            w1cp = ep(tc.tile_pool(name="w1c", bufs=3))
            h3p = ep(tc.tile_pool(name="h3p", bufs=1))
            x1relp = ep(tc.tile_pool(name="x1rel", bufs=2))
            outp = ep(tc.tile_pool(name="outp", bufs=1))
            mmps = ep(tc.tile_pool(name="mm_ps", bufs=3, space="PSUM"))
            zps = ep(tc.tile_pool(name="z_ps", bufs=1, space="PSUM"))
            f1ps = ep(tc.tile_pool(name="f1_ps", bufs=2, space="PSUM"))
            f2ps = ep(tc.tile_pool(name="f2_ps", bufs=2, space="PSUM"))
            # build block-diagonal kv and ksum tiles (scaled 1/KVS) from cco
            kv_stage = kv2p.tile([128, 8, 65], F32)
            kv_bd = kv2p.tile([128, 8, 128], F8)
            bd = kv2p.tile([128, 8, 16], F8)
            nc.vector.memset(kv_bd[:], 0.0)
            nc.vector.memset(bd[:], 0.0)
            for h in range(H):
                ti, hf, slot = h // 8, (h % 8) // 4, h % 4
                pb = (h % 2) * 64
                nc.sync.dma_start(
                    out=kv_stage[pb:pb + 64, h // 2, :],
                    in_=cco[ti, hf * 64:(hf + 1) * 64, slot, :])
                nc.vector.tensor_scalar_mul(
                    out=kv_bd[pb:pb + 64, h // 2, pb:pb + 64],
                    in0=kv_stage[pb:pb + 64, h // 2, 0:64], scalar1=1.0 / KVS)
                nc.vector.tensor_scalar_mul(
                    out=bd[pb:pb + 64, h // 2, h:h + 1],
                    in0=kv_stage[pb:pb + 64, h // 2, 64:65], scalar1=1.0 / KVS)

            h2T_tiles = {}

            def emit_attn_group(g):
                """z, attn, proj(+residual), LN2, h2T for group g."""
                gsl = slice(g * 512, (g + 1) * 512)
                # z = 1/(q . ksum/KVS + eps/KVS)
                zp = zps.tile([16, 512], F32, tag="z")
                for i in range(4):
                    nc.tensor.matmul(zp[:], lhsT=bd[:, 2 * i:2 * i + 2, :],
                                     rhs=qT[:, 2 * i:2 * i + 2, gsl],
                                     start=(i == 0), stop=(i == 3), perf_mode=DR)
                zf = ztp.tile([16, 512], F32, tag="zf")
                zb = ztp.tile([16, 512], BF16, tag="zb")
                nc.vector.tensor_scalar_add(out=zf[:], in0=zp[:], scalar1=EPS_ATTN / KVS)
                nc.vector.reciprocal(out=zb[:], in_=zf[:])
                nc.sync.dma_start(out=z_d[:, gsl], in_=zb[:])
                z_bc = zbcp.tile([128, 8, 512], BF16, tag="zbc")
                zd_ap = z_d[:]
                for sub in range(2):
                    nc.sync.dma_start(
                        out=z_bc[sub * 64:(sub + 1) * 64, :, :],
                        in_=bass.AP(tensor=zd_ap.tensor,
                                    offset=zd_ap.offset + sub * TOK + g * 512,
                                    ap=[[0, 64], [2 * TOK, 8], [1, 512]]))
                # attn_T[cc] = (kv_bd[cc]^T @ qT[cc]) * z
                attn_f8 = attnp.tile([128, 8, 512], F8, tag="attn")
                for cc in range(8):
                    aps = mmps.tile([128, 512], F32, tag="mm")
                    nc.tensor.matmul(aps[:], lhsT=kv_bd[:, cc, :],
                                     rhs=qT[:, cc, gsl], start=True, stop=True)
                    nc.vector.tensor_tensor(out=attn_f8[:, cc, :], in0=aps[:],
                                            in1=z_bc[:, cc, :], op=ALU.mult)
                # proj + residual -> x1; batched LN2 -> h2T group tile
                h2Tg = h2Tp.tile([128, 8, 512], BF16, tag="h2T", name=f"h2T{g}")
                mvq = lnp.tile([128, 4, 2], F32, tag="mvq")
                x1q = []
                for tl in range(4):
                    tt = g * 4 + tl
                    x_rel = xrelp.tile([128, C], F32, tag="xrel")
                    nc.sync.dma_start(out=x_rel[:], in_=xs_v[tt])
                    x1_t = x1tp.tile([128, C], F32, tag="x1")
                    for oc in range(2):
                        osl = slice(oc * 512, (oc + 1) * 512)
                        pps = mmps.tile([128, 512], F32, tag="mm")
                        for i in range(4):
                            nc.tensor.matmul(pps[:], lhsT=attn_f8[:, 2 * i:2 * i + 2, tl * 128:(tl + 1) * 128],
                                             rhs=wp_sb[:, 2 * i:2 * i + 2, osl],
                                             start=(i == 0), stop=(i == 3), perf_mode=DR)
                        nc.vector.scalar_tensor_tensor(
                            out=x1_t[:, osl], in0=pps[:], scalar=1.0 / WS,
                            in1=x_rel[:, osl], op0=ALU.mult, op1=ALU.add)
                        if has_bp:
                            nc.vector.tensor_tensor(out=x1_t[:, osl], in0=x1_t[:, osl],
                                                    in1=bp_bc[:, osl], op=ALU.add)
                    nc.sync.dma_start(out=x1s[tt], in_=x1_t[:])
                    _emit_ln_stats(nc, lnp, x1_t, mvq, tl)
                    x1q.append(x1_t)
                nrq = _emit_ln_nr(nc, lnp, mvq)
                for tl in range(4):
                    h2_t = h2locp.tile([128, C], BF16, tag="h2")
                    _emit_ln_apply(nc, nrq, tl, x1q[tl], h2_t)
                    # transpose straight into the group tile via the DMA XBAR
                    for cc in range(8):
                        nc.scalar.dma_start_transpose(
                            out=h2Tg[:, cc, tl * 128:(tl + 1) * 128],
                            in_=h2_t[:, cc * 128:(cc + 1) * 128])
                h2T_tiles[g] = h2Tg

            emit_attn_group(0)
            for g in range(NG):
                h2Tg = h2T_tiles.pop(g)
                # fc1 + gelu -> h3 (bf16, hid-major)
                h3 = h3p.tile([128, 32, 512], BF16, tag="h3", name=f"h3_{g}")
                w1pre = {}
                for hd in range(2):
                    w1c = w1cp.tile([128, 8, 128], BF16, tag="w1c", name=f"w1c{g}_{hd}")
                    nc.scalar.dma_start(out=w1c[:], in_=w1_v[:, :, hd * 128:(hd + 1) * 128])
                    w1pre[hd] = w1c
                for hd in range(32):
                    if hd in w1pre:
                        w1c = w1pre.pop(hd)
                    else:
                        w1c = w1cp.tile([128, 8, 128], BF16, tag="w1c", name=f"w1c{g}_{hd}")
                        nc.scalar.dma_start(out=w1c[:], in_=w1_v[:, :, hd * 128:(hd + 1) * 128])
                    fp = f1ps.tile([128, 512], F32, tag="f1")
                    for cc in range(8):
                        nc.tensor.matmul(fp[:], lhsT=w1c[:, cc, :], rhs=h2Tg[:, cc, :],
                                         start=(cc == 0), stop=(cc == 7))
                    if has_bg:
                        nc.scalar.activation(out=h3[:, hd, :], in_=fp[:], func=AF.Gelu,
                                             bias=bg_sb[:, hd:hd + 1], scale=1.0)
                    else:
                        nc.scalar.activation(out=h3[:, hd, :], in_=fp[:], func=AF.Gelu)
                # overlap next group's attention block with this group's fc2
                if g + 1 < NG:
                    emit_attn_group(g + 1)
                # fc2 + residual -> out
                for tl in range(4):
                    tt = g * 4 + tl
                    x1_rel = x1relp.tile([128, C], F32, tag="x1rel")
                    nc.sync.dma_start(out=x1_rel[:], in_=x1s[tt])
                    o_t = outp.tile([128, C], F32, tag="ot")
                    for oc in range(2):
                        osl = slice(oc * 512, (oc + 1) * 512)
                        fp2 = f2ps.tile([128, 512], F32, tag="f2")
                        for hd in range(32):
                            nc.tensor.matmul(fp2[:], lhsT=h3[:, hd, tl * 128:(tl + 1) * 128],
                                             rhs=w2_sb[:, hd, osl],
                                             start=(hd == 0), stop=(hd == 31))
                        nc.vector.tensor_tensor(out=o_t[:, osl], in0=fp2[:],
                                                in1=x1_rel[:, osl], op=ALU.add)
                        if has_b2:
                            nc.vector.tensor_tensor(out=o_t[:, osl], in0=o_t[:, osl],
                                                    in1=b2_bc[:, osl], op=ALU.add)
                    nc.sync.dma_start(out=out_v[tt], in_=o_t[:])

    nc.compile()
    return nc


def _prep_inputs(x, norm1_g, norm1_b, qkv_w, proj_w, proj_b, norm2_g, norm2_b,
                 fc1_w, fc1_b, fc2_w, fc2_b):
    """Host-side weight prep: fold LN gains into weights, LN biases into
    per-output biases; quantize attention weights to fp8 (x32) and MLP
    weights to bf16. Returns (flags, per-core in_maps)."""
    F8NP = ml_dtypes.float8_e4m3
    BFNP = ml_dtypes.bfloat16
    x = np.asarray(x, np.float32)
    g1 = np.asarray(norm1_g, np.float32)
    b1 = np.asarray(norm1_b, np.float32)
    qkv_w = np.asarray(qkv_w, np.float32)
    proj_w = np.asarray(proj_w, np.float32)
    proj_b = np.asarray(proj_b, np.float32)
    g2 = np.asarray(norm2_g, np.float32)
    b2 = np.asarray(norm2_b, np.float32)
    fc1_w = np.asarray(fc1_w, np.float32)
    fc1_b = np.asarray(fc1_b, np.float32)
    fc2_w = np.asarray(fc2_w, np.float32)
    fc2_b = np.asarray(fc2_b, np.float32)

    def f8(w):
        return np.clip(w * WS, -440.0, 440.0).astype(F8NP)

    wq_t = f8(np.ascontiguousarray((qkv_w[0:C] * g1[None, :]).T))
    wkv_t = f8(np.ascontiguousarray((qkv_w[C:3 * C] * g1[None, :]).T))
    wp_t = f8(np.ascontiguousarray(proj_w.T))
    w1_t = np.ascontiguousarray((fc1_w * g2[None, :]).T).astype(BFNP)
    w2_t = np.ascontiguousarray(fc2_w.T).astype(BFNP)
    bq_v = (qkv_w[0:C] @ b1).astype(np.float32)
    bk_v = (qkv_w[C:2 * C] @ b1).astype(np.float32)
    bv_v = (qkv_w[2 * C:3 * C] @ b1).astype(np.float32)
    bg_v = (fc1_w @ b2 + fc1_b).astype(np.float32)

    flags = (bool(np.any(bq_v)), bool(np.any(bk_v)), bool(np.any(bv_v)),
             bool(np.any(bg_v)), bool(np.any(proj_b)), bool(np.any(fc2_b)))

    shared = dict(wq=wq_t, wkv=wkv_t, wp=wp_t, w1=w1_t, w2=w2_t,
                  bq=np.ascontiguousarray(bq_v), bk=np.ascontiguousarray(bk_v),
                  bv=np.ascontiguousarray(bv_v), bg=np.ascontiguousarray(bg_v),
                  bp=proj_b, b2o=fc2_b)
    in_maps = []
    for core in range(8):
        b, half = core // 2, core % 2
        xs = np.ascontiguousarray(x[b, half * TOK:(half + 1) * TOK, :])
        in_maps.append({"xs": xs, **shared})
    return flags, in_maps


def get_compiled(flags):
    if flags not in _BUILD_CACHE:
        _BUILD_CACHE[flags] = _build(flags)
    return _BUILD_CACHE[flags]


def kernel(**inputs) -> np.ndarray:
    flags, in_maps = _prep_inputs(**inputs)
    nc = get_compiled(flags)
    res = run_bass_kernel_spmd(nc, in_maps=in_maps, core_ids=list(range(8)))
    shards = [res.results[c]["out"] for c in range(8)]
    full = np.empty((B, N, C), np.float32)
    for core in range(8):
        b, half = core // 2, core % 2
        full[b, half * TOK:(half + 1) * TOK, :] = shards[core]
    return full
